# revision 16
# baseline (speedup 1.0000x reference)
"""ModalityUntiedAttention on 8 TRN2 NeuronCores (Bass/Tile).

Sharding: data-parallel over batch (cores 0-3 -> batch 0, cores 4-7 -> batch 1),
tensor-parallel over heads within each 4-core group (4 q heads + 2 kv heads per
core).

Expert (modality) routing: tokens are sorted by modality WITHIN each 512-token
attention group (host-side permutation). Attention stays exact: group pairs
below the diagonal are fully causal-allowed; the in-group (diagonal) causal
masks are host-computed for the permuted order; the host un-permutes the
output rows. With >=128 tokens of each modality per group, each group splits
into [pure0, mixed, mixed, pure1] 128-token tiles: pure tiles run one expert's
weights; mixed tiles accumulate W0@x + Wd@(m*x) in one PSUM chain (Wd=W1-W0,
m*x premasked on host), so no select is needed at eviction.

Attention computes scores^T (keys on partitions), exp without max subtraction,
denominators via ones-column matmuls batched per (group, head) so the PE never
alternates tile shapes (shape flips cost ~120ns/matmul), and the denominator
reciprocal is broadcast with GPSIMD instead of a PE matmul. The wo projection
accumulates the W0 and Wd passes into a single PSUM bank (ofT premultiplied by
the modality mask on DVE), is evicted with a plain scalar copy, and is
ReduceScattered (bf16) over each 4-core group in 256-token sub-chunks so the
final chunk's collective tail is halved.
"""
import sys

sys.path.insert(0, '/opt/trn_rl_repo')

import os
from contextlib import ExitStack

import numpy as np
import ml_dtypes

import concourse.bass as bass
import concourse.tile as tile
from concourse import bacc, mybir
from concourse.bass import ts, ds, _add_dep_helper
from concourse.bass_utils import run_bass_kernel_spmd
from concourse.masks import make_identity

F32 = mybir.dt.float32
BF16 = mybir.dt.bfloat16

E = 2
HQ = 16
HK = 8
HD = 128
DIM = 2048
BS = 2
SEQ = 2048
EPS = 1e-6

N_CORES = 8
TP = 4                     # cores per batch group
HQC = HQ // TP             # 4 q heads per core
HKC = HK // TP             # 2 kv heads per core
DQ = HQC * HD              # 512 q cols per core
DKV = HKC * HD             # 256 k (and v) cols per core
NT = SEQ // 128            # 16 token tiles
KT = DIM // 128            # 16 contraction tiles
NG = 4                     # 512-token attention groups
NS = 2                     # reduce-scatter sub-chunks per group (256 tokens)
GROUPS = [[0, 1, 2, 3], [4, 5, 6, 7]]

_BUILD_CACHE = {}

MUL = mybir.AluOpType.mult
ADD = mybir.AluOpType.add


def build_nc(has_qkw: bool, has_anw: bool, kinds: tuple):
    """kinds[T] in {0: pure expert-0, 1: pure expert-1, 2: mixed}."""
    mix_ids = [T for T, k in enumerate(kinds) if k == 2]
    mix_pos = {T: i for i, T in enumerate(mix_ids)}
    nmix = max(1, len(mix_ids))

    nc = bacc.Bacc("TRN2", target_bir_lowering=False, debug=False,
                   num_devices=N_CORES)

    xT = nc.dram_tensor("xT", [NT, 128, KT, 128], BF16, kind="ExternalInput")
    xmT = nc.dram_tensor("xmT", [nmix, 128, KT, 128], BF16, kind="ExternalInput")
    w0 = nc.dram_tensor("w0", [DIM, DQ + 2 * DKV], BF16, kind="ExternalInput")
    w1 = nc.dram_tensor("w1", [DIM, DQ + 2 * DKV], BF16, kind="ExternalInput")
    wd = nc.dram_tensor("wd", [DIM, DQ + 2 * DKV], BF16, kind="ExternalInput")
    wo0 = nc.dram_tensor("wo0", [DQ, DIM], BF16, kind="ExternalInput")
    wo1 = nc.dram_tensor("wo1", [DQ, DIM], BF16, kind="ExternalInput")
    wod = nc.dram_tensor("wod", [DQ, DIM], BF16, kind="ExternalInput")
    cosf = nc.dram_tensor("cosf", [SEQ, HD], F32, kind="ExternalInput")
    sinf = nc.dram_tensor("sinf", [SEQ, HD], F32, kind="ExternalInput")
    mrow = nc.dram_tensor("mrow", [1, SEQ], BF16, kind="ExternalInput")
    dmin = nc.dram_tensor("dmin", [NT, 128, 512], BF16, kind="ExternalInput")
    if has_qkw:
        qkw = nc.dram_tensor("qkw", [SEQ, DQ + DKV], F32, kind="ExternalInput")
    if has_anw:
        anw0 = nc.dram_tensor("anw0", [1, DIM], F32, kind="ExternalInput")
        anwd = nc.dram_tensor("anwd", [1, DIM], F32, kind="ExternalInput")
        mfin = nc.dram_tensor("mfin", [64, (NG - 1) * NS + 4], F32,
                              kind="ExternalInput")

    out_dram = nc.dram_tensor("out", [SEQ // 4, DIM], F32, kind="ExternalOutput")

    with tile.TileContext(nc) as tc:
        with ExitStack() as ctx:
            const = ctx.enter_context(tc.tile_pool(name="const", bufs=1))
            persist = ctx.enter_context(tc.tile_pool(name="persist", bufs=1))
            dram = ctx.enter_context(tc.tile_pool(name="dram", bufs=1, space="DRAM"))

            ident_bf = const.tile([128, 128], BF16)
            make_identity(nc, ident_bf[:])
            ones_f = const.tile([128, 1], F32)
            nc.gpsimd.memset(ones_f[:], 1.0)
            ones_col = const.tile([128, 1], BF16)
            nc.scalar.copy(ones_col[:], ones_f[:])
            eps_q = const.tile([128, 1], F32)
            nc.gpsimd.memset(eps_q[:], float(128.0 * EPS))
            eps_1 = const.tile([128, 1], F32)
            nc.gpsimd.memset(eps_1[:], float(EPS))
            mrow_sb = const.tile([1, SEQ], BF16)
            nc.sync.dma_start(mrow_sb[:], mrow[:, :])
            m_ball = const.tile([128, SEQ], BF16)
            nc.gpsimd.partition_broadcast(m_ball[:], mrow_sb[:])
            dmasks = const.tile([128, NT, 512], BF16)

            # persistent activation buffers (bf16)
            QT = persist.tile([128, HQC, SEQ], BF16)    # q^T per head (hd, tok)
            KTb = persist.tile([128, HKC, SEQ], BF16)   # k^T per kv head
            Vb = persist.tile([128, NT, DKV], BF16)     # v natural (tok, hd)

            # ------------- Phase 1: QKV projection + norms + rope ------------
            with ExitStack() as p1:
                wpool = p1.enter_context(tc.tile_pool(name="wpool", bufs=1))
                ropep = p1.enter_context(tc.tile_pool(name="ropep", bufs=1))
                xpool = p1.enter_context(tc.tile_pool(name="xpool", bufs=3))
                xmpool = p1.enter_context(tc.tile_pool(name="xmpool", bufs=2))
                qkps = p1.enter_context(tc.tile_pool(name="qkps", bufs=6, space="PSUM"))
                tps = p1.enter_context(tc.tile_pool(name="tps", bufs=2, space="PSUM"))
                work = p1.enter_context(tc.tile_pool(name="work", bufs=2))
                if has_qkw:
                    qkwpool = p1.enter_context(tc.tile_pool(name="qkwpool", bufs=2))

                w0_sb = wpool.tile([128, KT, DQ + 2 * DKV], BF16)
                w1_sb = wpool.tile([128, KT, DQ + 2 * DKV], BF16)
                wd_sb = wpool.tile([128, KT, DQ + 2 * DKV], BF16)
                w0_r = w0.ap().rearrange("(k p) f -> p k f", p=128)
                w1_r = w1.ap().rearrange("(k p) f -> p k f", p=128)
                wd_r = wd.ap().rearrange("(k p) f -> p k f", p=128)
                # w0 feeds the first tile, wd the first mixed tile (T=1),
                # w1 only the first pure-1 tile (T=3): load in that order
                for k in range(KT):
                    nc.gpsimd.dma_start(w0_sb[:, k, :], w0_r[:, k, :])
                for k in range(KT):
                    nc.gpsimd.dma_start(wd_sb[:, k, :], wd_r[:, k, :])
                for k in range(KT):
                    nc.gpsimd.dma_start(w1_sb[:, k, :], w1_r[:, k, :])
                cos_sb = ropep.tile([128, NT, HD], F32)
                nc.gpsimd.dma_start(cos_sb[:], cosf.ap().rearrange("(t p) d -> p t d", p=128))
                sin_sb = ropep.tile([128, NT, HD], F32)
                nc.gpsimd.dma_start(sin_sb[:], sinf.ap().rearrange("(t p) d -> p t d", p=128))

                for T in range(NT):
                    kind = kinds[T]
                    xt = xpool.tile([128, KT, 128], BF16, tag="xt")
                    nc.sync.dma_start(xt[:], xT.ap()[T])
                    if kind == 2:
                        xm = xmpool.tile([128, KT, 128], BF16, tag="xm")
                        nc.sync.dma_start(xm[:], xmT.ap()[mix_pos[T]])

                    wa_sb = w1_sb if kind == 1 else w0_sb
                    pa_q = qkps.tile([128, 512], F32, tag="qk")
                    pa_kv = qkps.tile([128, 512], F32, tag="qk")
                    for k in range(KT):
                        st = k == 0
                        sp = (k == KT - 1) and kind != 2
                        lhsT = xt[:, k, :]
                        nc.tensor.matmul(pa_q[:], lhsT, wa_sb[:, k, 0:512], start=st, stop=sp)
                        nc.tensor.matmul(pa_kv[:], lhsT, wa_sb[:, k, 512:1024], start=st, stop=sp)
                    if kind == 2:
                        # delta pass: accumulate Wd @ (m*x) into the same psum
                        for k in range(KT):
                            sp = k == KT - 1
                            lhsT = xm[:, k, :]
                            nc.tensor.matmul(pa_q[:], lhsT, wd_sb[:, k, 0:512], start=False, stop=sp)
                            nc.tensor.matmul(pa_kv[:], lhsT, wd_sb[:, k, 512:1024], start=False, stop=sp)
                    q_src, kv_src = pa_q, pa_kv

                    # v: plain evict (bf16)
                    nc.vector.tensor_copy(Vb[:, T, :], kv_src[:, 256:512])

                    # rms stats via ACT Square + accum; eps folded into Sqrt
                    msq_q = work.tile([128, 4], F32, tag="msq_q")
                    scr = work.tile([128, 128], F32, tag="scr")
                    for h in range(HQC):
                        nc.scalar.activation(
                            scr[:], q_src[:, ts(h, 128)],
                            mybir.ActivationFunctionType.Square,
                            accum_out=msq_q[:, h:h + 1])
                    msq_k = work.tile([128, 2], F32, tag="msq_k")
                    for h in range(HKC):
                        nc.scalar.activation(
                            scr[:], kv_src[:, ts(h, 128)],
                            mybir.ActivationFunctionType.Square,
                            accum_out=msq_k[:, h:h + 1])
                    sq_q = work.tile([128, 4], F32, tag="sq_q")
                    nc.scalar.activation(sq_q[:], msq_q[:],
                                         mybir.ActivationFunctionType.Sqrt,
                                         scale=1.0, bias=eps_q[:])
                    rs_q = work.tile([128, 4], F32, tag="rs_q")
                    nc.vector.reciprocal_approx_fast(rs_q[:], sq_q[:])
                    sq_k = work.tile([128, 2], F32, tag="sq_k")
                    nc.scalar.activation(sq_k[:], msq_k[:],
                                         mybir.ActivationFunctionType.Sqrt,
                                         scale=1.0 / 128.0, bias=eps_1[:])
                    rs_k = work.tile([128, 2], F32, tag="rs_k")
                    nc.vector.reciprocal_approx_fast(rs_k[:], sq_k[:])

                    if has_qkw:
                        qkw_t = qkwpool.tile([128, DQ + DKV], F32, tag="qkw")
                        nc.sync.dma_start(qkw_t[:], qkw.ap()[ts(T, 128), :])
                        q_w = work.tile([128, 512], F32, tag="q_w")
                        nc.vector.tensor_mul(q_w[:], q_src[:, 0:512], qkw_t[:, 0:DQ])
                        k_w = work.tile([128, 256], F32, tag="k_w")
                        nc.vector.tensor_mul(k_w[:], kv_src[:, 0:256], qkw_t[:, DQ:DQ + DKV])
                        q_src, kv_src = q_w, k_w

                    # rope (de-interleaved hd: [even dims | odd dims])
                    cos_t = cos_sb[:, T, :]
                    sin_t = sin_sb[:, T, :]

                    def rope(dst, src, rs, h):
                        base = src[:, ts(h, 128)]
                        t1 = work.tile([128, 128], BF16, tag="rope_t1")
                        nc.vector.scalar_tensor_tensor(
                            out=t1[:], in0=base, scalar=rs[:, h:h + 1], in1=cos_t,
                            op0=MUL, op1=MUL)
                        t2 = work.tile([128, 128], BF16, tag="rope_t2")
                        nc.vector.scalar_tensor_tensor(
                            out=t2[:, 0:64], in0=base[:, 64:128], scalar=rs[:, h:h + 1],
                            in1=sin_t[:, 0:64], op0=MUL, op1=MUL)
                        nc.vector.scalar_tensor_tensor(
                            out=t2[:, 64:128], in0=base[:, 0:64], scalar=rs[:, h:h + 1],
                            in1=sin_t[:, 64:128], op0=MUL, op1=MUL)
                        nc.vector.tensor_add(dst[:, ts(h, 128)], t1[:], t2[:])

                    q_rot = work.tile([128, 512], BF16, tag="q_rot")
                    for h in range(HQC):
                        rope(q_rot, q_src, rs_q, h)
                    k_rot = work.tile([128, 256], BF16, tag="k_rot")
                    for h in range(HKC):
                        rope(k_rot, kv_src, rs_k, h)

                    # transpose to (hd, tok) layouts (bf16: 1 cycle/row)
                    for h in range(HQC):
                        tp = tps.tile([128, 128], BF16, tag="tp")
                        nc.tensor.transpose(tp[:], q_rot[:, ts(h, 128)], ident_bf[:])
                        nc.scalar.copy(QT[:, h, ts(T, 128)], tp[:])
                    for h in range(HKC):
                        tp = tps.tile([128, 128], BF16, tag="tp")
                        nc.tensor.transpose(tp[:], k_rot[:, ts(h, 128)], ident_bf[:])
                        nc.scalar.copy(KTb[:, h, ts(T, 128)], tp[:])

            # ------------- Phase 2+3: attention + wo + RS + final norm -------
            with ExitStack() as p23:
                wopool = p23.enter_context(tc.tile_pool(name="wopool", bufs=1))
                ofp = p23.enter_context(tc.tile_pool(name="ofp", bufs=1))
                sps = p23.enter_context(tc.tile_pool(name="sps", bufs=2, space="PSUM"))
                otps = p23.enter_context(tc.tile_pool(name="otps", bufs=1, space="PSUM"))
                dnps = p23.enter_context(tc.tile_pool(name="dnps", bufs=1, space="PSUM"))
                wops = p23.enter_context(tc.tile_pool(name="wops", bufs=2, space="PSUM"))
                probs = p23.enter_context(tc.tile_pool(name="probs", bufs=8))
                att = p23.enter_context(tc.tile_pool(name="att", bufs=2))
                opool = p23.enter_context(tc.tile_pool(name="opool", bufs=2))
                npool = p23.enter_context(tc.tile_pool(name="npool", bufs=2))

                ofT = ofp.tile([128, HQC, SEQ], BF16)   # out_flat^T (hd, tok)
                nc.gpsimd.dma_start(dmasks[:], dmin.ap().rearrange("t p f -> p t f"))

                wo0_sb = wopool.tile([128, 4, DIM], BF16)
                nc.sync.dma_start(wo0_sb[:], wo0.ap().rearrange("(k p) f -> p k f", p=128))
                wo1_sb = wopool.tile([128, 4, DIM], BF16)
                nc.sync.dma_start(wo1_sb[:], wo1.ap().rearrange("(k p) f -> p k f", p=128))
                wod_sb = wopool.tile([128, 4, DIM], BF16)
                nc.sync.dma_start(wod_sb[:], wod.ap().rearrange("(k p) f -> p k f", p=128))
                if has_anw:
                    anw0_sb = wopool.tile([1, DIM], F32)
                    nc.sync.dma_start(anw0_sb[:], anw0[:, :])
                    anwd_sb = wopool.tile([1, DIM], F32)
                    nc.sync.dma_start(anwd_sb[:], anwd[:, :])
                    anw0_b = wopool.tile([128, DIM], F32)
                    nc.gpsimd.partition_broadcast(anw0_b[:], anw0_sb[:])
                    anwd_b = wopool.tile([128, DIM], F32)
                    nc.gpsimd.partition_broadcast(anwd_b[:], anwd_sb[:])
                    mfin_sb = wopool.tile([64, (NG - 1) * NS + 4], F32)
                    nc.sync.dma_start(mfin_sb[:], mfin[:, :])

                pending_rs = []

                def do_final_norm(g, s, ns, ci, rs_out, dep=None):
                    # norm DMAs ride the scalar HWDGE queue: a sync-queue load
                    # waiting on the RS semaphore would block later wo stages
                    p = 128 // ns
                    sum_t = npool.tile([64, DIM], BF16, tag="sum_sb")
                    sum_sb = sum_t[0:p, :]
                    first = nc.scalar.dma_start(sum_sb, rs_out[:])
                    if dep is not None:
                        _add_dep_helper(first.ins, dep.ins, sync=False,
                                        reason="defer norm past next chunk")
                    fin_t = npool.tile([64, DIM], F32, tag="fin")
                    fin = fin_t[0:p, :]
                    z = npool.tile([64, 1], F32, tag="z")
                    nc.vector.scalar_tensor_tensor(
                        out=fin, in0=sum_sb, scalar=1.0, in1=sum_sb,
                        op0=MUL, op1=MUL, accum_out=z[0:p, :])
                    sz = npool.tile([64, 1], F32, tag="sz")
                    nc.scalar.activation(sz[0:p, :], z[0:p, :],
                                         mybir.ActivationFunctionType.Sqrt,
                                         scale=1.0 / float(DIM), bias=eps_1[0:p, :])
                    rz = npool.tile([64, 1], F32, tag="rz")
                    nc.vector.reciprocal_approx_fast(rz[0:p, :], sz[0:p, :])
                    nc.scalar.mul(fin, sum_sb, rz[0:p, :])
                    if has_anw:
                        anw_t = npool.tile([64, DIM], F32, tag="anw_sel")
                        anw_sel = anw_t[0:p, :]
                        nc.vector.scalar_tensor_tensor(
                            out=anw_sel, in0=anwd_b[0:p, :],
                            scalar=mfin_sb[0:p, ci:ci + 1],
                            in1=anw0_b[0:p, :], op0=MUL, op1=ADD)
                        nc.vector.tensor_mul(fin, fin, anw_sel)
                    nc.scalar.dma_start(
                        out_dram.ap()[ds(128 * g + p * s, p), :], fin)

                sub_ci = 0
                for g in range(NG):
                    njt = 4 * (g + 1)
                    # in-group (masked) tiles first: their exp+mask latency is
                    # covered by the later tiles' scores matmuls
                    jlist = list(range(4 * g, njt)) + list(range(0, 4 * g))
                    jpairs = [(jlist[2 * i], jlist[2 * i + 1])
                              for i in range(njt // 2)]
                    for h in range(HQC):
                        kv = h // (HQC // HKC)
                        ot_ps = otps.tile([128, 512], F32, tag="ot")
                        dn_ps = dnps.tile([1, 512], F32, tag="dn")
                        ptiles = []
                        pend_av = None
                        for jp, (j0, j1) in enumerate(jpairs):
                            s_ps = sps.tile([128, 2, 512], F32, tag="s")
                            for dj, j in enumerate((j0, j1)):
                                nc.tensor.matmul(
                                    s_ps[:, dj, :], KTb[:, kv, ts(j, 128)],
                                    QT[:, h, ts(g, 512)], start=True, stop=True)
                            p_t = probs.tile([128, 2, 512], BF16, tag="p")
                            nc.scalar.activation(
                                p_t[:], s_ps[:], mybir.ActivationFunctionType.Exp)
                            if j0 >= 4 * g:
                                pm_t = probs.tile([128, 2, 512], BF16, tag="pm")
                                nc.vector.tensor_mul(
                                    pm_t[:], p_t[:], dmasks[:, ds(j0, 2), :])
                                p_t = pm_t
                            ptiles.append(p_t)
                            # AV for the previous pair (software pipeline: the
                            # scores above already covered this pair's exp)
                            if pend_av is not None:
                                pv_t, (pj0, pj1), pjp = pend_av
                                for dj, j in enumerate((pj0, pj1)):
                                    nc.tensor.matmul(
                                        ot_ps[:], Vb[:, j, ts(kv, 128)],
                                        pv_t[:, dj, :],
                                        start=(pjp == 0 and dj == 0), stop=False)
                            pend_av = (p_t, (j0, j1), jp)
                        pv_t, (pj0, pj1), pjp = pend_av
                        for dj, j in enumerate((pj0, pj1)):
                            nc.tensor.matmul(
                                ot_ps[:], Vb[:, j, ts(kv, 128)], pv_t[:, dj, :],
                                start=(pjp == 0 and dj == 0), stop=(dj == 1))
                        # batched denominator matmuls (single tile shape run)
                        for jp, p_t in enumerate(ptiles):
                            for dj in range(2):
                                nc.tensor.matmul(
                                    dn_ps[:], ones_col[:], p_t[:, dj, :],
                                    start=(jp == 0 and dj == 0),
                                    stop=(jp == len(ptiles) - 1 and dj == 1))
                        # raw evict; then normalize ofT via gpsimd broadcast
                        nc.vector.tensor_copy(ofT[:, h, ts(g, 512)], ot_ps[:])
                        den = att.tile([1, 512], F32, tag="den")
                        nc.vector.reciprocal_approx_fast(den[:], dn_ps[:])
                        den_b = att.tile([128, 512], F32, tag="den_b")
                        nc.gpsimd.partition_broadcast(den_b[:], den[:])
                        nc.vector.tensor_mul(
                            ofT[:, h, ts(g, 512)], ofT[:, h, ts(g, 512)], den_b[:])

                    # wo projection for this 512-token chunk; each sub-chunk
                    # gets its own staging tile so its RS fires as soon as its
                    # tiles are evicted. The last group uses 128-token subs:
                    # its final RS is fully tail-exposed, so keep it small.
                    ns = NS if g < NG - 1 else 4
                    for s in range(ns):
                        rs_in = dram.tile([512 // ns, DIM], BF16,
                                          tag=f"rs_in{ns}", bufs=2 * ns)
                        for u in range((4 // ns) * s, (4 // ns) * (s + 1)):
                            T = 4 * g + u
                            kind = kinds[T]
                            o_sb = opool.tile([128, DIM], BF16, tag="o_sb")
                            if kind == 2:
                                ofm = opool.tile([128, 4, 128], BF16, tag="ofm")
                                for kk in range(4):
                                    nc.vector.tensor_mul(
                                        ofm[:, kk, :], ofT[:, kk, ts(T, 128)],
                                        m_ball[:, ts(T, 128)])
                            woa_sb = wo1_sb if kind == 1 else wo0_sb
                            for n in range(4):
                                wo_ps = wops.tile([128, 512], F32, tag="wop")
                                for kk in range(4):
                                    nc.tensor.matmul(
                                        wo_ps[:], ofT[:, kk, ts(T, 128)],
                                        woa_sb[:, kk, ts(n, 512)],
                                        start=(kk == 0), stop=(kind != 2 and kk == 3))
                                if kind == 2:
                                    for kk in range(4):
                                        nc.tensor.matmul(
                                            wo_ps[:], ofm[:, kk, :],
                                            wod_sb[:, kk, ts(n, 512)],
                                            start=False, stop=(kk == 3))
                                nc.scalar.copy(o_sb[:, ts(n, 512)], wo_ps[:])
                            last_rsin_dma = nc.sync.dma_start(
                                rs_in[ts(u - (4 // ns) * s, 128), :], o_sb[:])

                        rs_out = dram.tile([512 // ns // 4, DIM], BF16,
                                           tag=f"rs_out{ns}", bufs=2 * ns)
                        nc.gpsimd.collective_compute(
                            "ReduceScatter", mybir.AluOpType.add,
                            replica_groups=GROUPS,
                            ins=[rs_in.opt()], outs=[rs_out.opt()])
                        pending_rs.append((g, s, ns, sub_ci, rs_out))
                        sub_ci += 1
                        # final norm for an older chunk: its RS finished while
                        # this chunk computed, so the queues never block on it
                        while len(pending_rs) > 2:
                            pg, ps, pns, pci, prs = pending_rs.pop(0)
                            do_final_norm(pg, ps, pns, pci, prs,
                                          dep=last_rsin_dma)

                for pg, ps, pns, pci, prs in pending_rs:
                    do_final_norm(pg, ps, pns, pci, prs)

    nc.compile()
    return nc


def _plan(modality_ids):
    """Per-group stable modality sort; unified tile-kind plan across batches."""
    mids = np.asarray(modality_ids).reshape(BS, SEQ)
    perms = np.empty((BS, SEQ), np.int64)   # permuted pos -> original token idx
    sortable = True
    for b in range(BS):
        for G in range(NG):
            mg = mids[b, 512 * G:512 * (G + 1)]
            i0 = np.where(mg == 0)[0]
            i1 = np.where(mg == 1)[0]
            if len(i0) < 128 or len(i1) < 128:
                sortable = False
            perms[b, 512 * G:512 * (G + 1)] = 512 * G + np.concatenate([i0, i1])
    if sortable:
        kinds = tuple([0, 2, 2, 1] * NG)
    else:
        perms = np.tile(np.arange(SEQ), (BS, 1))
        kinds = tuple([2] * NT)
    return perms, kinds


def _prep_inputs(x, freqs_cos, freqs_sin, wq, wk, wv, wo,
                 q_norm_w, k_norm_w, attn_norm_w, modality_ids,
                 has_qkw, has_anw, perms, kinds):
    """Build the 8 per-core input maps (numpy marshaling only)."""
    x = np.asarray(x, np.float32)
    freqs_cos = np.asarray(freqs_cos, np.float32)
    freqs_sin = np.asarray(freqs_sin, np.float32)
    wq = np.asarray(wq, np.float32)
    wk = np.asarray(wk, np.float32)
    wv = np.asarray(wv, np.float32)
    wo = np.asarray(wo, np.float32)
    mids = np.asarray(modality_ids).reshape(BS, SEQ)
    mix_ids = [T for T, k in enumerate(kinds) if k == 2]

    # de-interleave the hd dimension: [even dims, odd dims]
    perm_hd = np.concatenate([np.arange(0, HD, 2), np.arange(1, HD, 2)])

    def permute_heads(w, nh):
        w4 = w.reshape(E, DIM, nh, HD)
        return w4[:, :, :, perm_hd].reshape(E, DIM, nh * HD)

    wq_p = permute_heads(wq, HQ)
    wk_p = permute_heads(wk, HK)
    wv_p = permute_heads(wv, HK)
    wo4 = wo.reshape(E, HQ, HD, DIM)[:, :, perm_hd, :].reshape(E, HQ * HD, DIM)

    cosf = np.concatenate([freqs_cos, freqs_cos], axis=1)          # (SEQ, HD)
    sinf = np.concatenate([-freqs_sin, freqs_sin], axis=1)         # (SEQ, HD)

    in_maps = []
    for c in range(N_CORES):
        b, r = divmod(c, TP)
        P = perms[b]
        qs = slice(r * DQ, (r + 1) * DQ)
        ks = slice(r * DKV, (r + 1) * DKV)
        w0c = np.concatenate([wq_p[0][:, qs], wk_p[0][:, ks], wv_p[0][:, ks]], axis=1)
        w1c = np.concatenate([wq_p[1][:, qs], wk_p[1][:, ks], wv_p[1][:, ks]], axis=1)
        m = mids[b].astype(np.float32)[P]
        # in-group causal masks for the permuted order
        pos = (P % 512)
        dmv = np.zeros((NT, 128, 512), np.float32)
        for j in range(NT):
            gj = j // 4
            kpos = pos[128 * j:128 * (j + 1)]
            qpos = pos[512 * gj:512 * (gj + 1)]
            dmv[j] = (kpos[:, None] <= qpos[None, :])
        xTc = np.ascontiguousarray(
            x[b].T[:, P].reshape(KT, 128, NT, 128).transpose(2, 1, 0, 3))
        xmTc = np.ascontiguousarray(
            xTc[mix_ids] * m.reshape(NT, 128)[mix_ids][:, None, None, :]
        ) if mix_ids else np.zeros((1, 128, KT, 128), np.float32)
        wo0c = wo4[0][r * DQ:(r + 1) * DQ, :]
        wo1c = wo4[1][r * DQ:(r + 1) * DQ, :]
        im = {
            "xT": xTc.astype(ml_dtypes.bfloat16),
            "xmT": xmTc.astype(ml_dtypes.bfloat16),
            "w0": w0c.astype(ml_dtypes.bfloat16),
            "w1": w1c.astype(ml_dtypes.bfloat16),
            "wd": (w1c - w0c).astype(ml_dtypes.bfloat16),
            "wo0": wo0c.astype(ml_dtypes.bfloat16),
            "wo1": wo1c.astype(ml_dtypes.bfloat16),
            "wod": (wo1c - wo0c).astype(ml_dtypes.bfloat16),
            "cosf": np.ascontiguousarray(cosf[P]),
            "sinf": np.ascontiguousarray(sinf[P]),
            "mrow": m.reshape(1, SEQ).astype(ml_dtypes.bfloat16),
            "dmin": dmv.astype(ml_dtypes.bfloat16),
        }
        if has_qkw:
            qw = np.asarray(q_norm_w, np.float32)[:, perm_hd]
            kw = np.asarray(k_norm_w, np.float32)[:, perm_hd]
            qsel = qw[mids[b][P]]
            ksel = kw[mids[b][P]]
            im["qkw"] = np.concatenate(
                [np.tile(qsel, (1, HQC)), np.tile(ksel, (1, HKC))], axis=1)
        if has_anw:
            aw = np.asarray(attn_norm_w, np.float32)
            im["anw0"] = np.ascontiguousarray(aw[0:1])
            im["anwd"] = (aw[1] - aw[0]).reshape(1, DIM).copy()
            mf = np.zeros((64, (NG - 1) * NS + 4), np.float32)
            ci = 0
            for g in range(NG):
                ns = NS if g < NG - 1 else 4
                p = 128 // ns
                for s in range(ns):
                    t0 = 512 * g + (512 // ns) * s + p * r
                    mf[:p, ci] = m[t0:t0 + p]
                    ci += 1
            im["mfin"] = mf
        in_maps.append(im)
    return in_maps


def kernel(**inputs):
    q_norm_w = np.asarray(inputs["q_norm_w"], np.float32)
    k_norm_w = np.asarray(inputs["k_norm_w"], np.float32)
    attn_norm_w = np.asarray(inputs["attn_norm_w"], np.float32)
    has_qkw = not (np.all(q_norm_w == 1.0) and np.all(k_norm_w == 1.0))
    has_anw = not np.all(attn_norm_w == 1.0)

    perms, kinds = _plan(inputs["modality_ids"])
    key = (has_qkw, has_anw, kinds)
    if key not in _BUILD_CACHE:
        _BUILD_CACHE[key] = build_nc(has_qkw, has_anw, kinds)
    nc = _BUILD_CACHE[key]

    in_maps = _prep_inputs(
        inputs["x"], inputs["freqs_cos"], inputs["freqs_sin"],
        inputs["wq"], inputs["wk"], inputs["wv"], inputs["wo"],
        q_norm_w, k_norm_w, attn_norm_w, inputs["modality_ids"],
        has_qkw, has_anw, perms, kinds)

    res = run_bass_kernel_spmd(nc, in_maps, core_ids=list(range(N_CORES)))

    out = np.empty((BS, SEQ, DIM), np.float32)
    for c in range(N_CORES):
        b, r = divmod(c, TP)
        P = perms[b]
        oc = res.results[c]["out"]          # (SEQ//4, DIM), permuted rows
        for g in range(NG):
            ns = NS if g < NG - 1 else 4
            p = 128 // ns
            for s in range(ns):
                t0 = 512 * g + (512 // ns) * s + p * r   # permuted positions
                o0 = 128 * g + p * s
                out[b, P[t0:t0 + p], :] = oc[o0:o0 + p, :]
    return out
